# revision 51
# baseline (speedup 1.0000x reference)
"""Trainium2 Bass kernel for the AggregateLayer pooling problem.

reference semantics (per batch b):
    dot_w[j] = <pref[b,j,:], c[b,0,:]>                      (j = 0..63)
    t_w[j]   = 1 / |t_pref[b,0,j] - t_c[b,0]|
    w        = softmax(dot_w + t_w)                          (over j)
    u[b,0,:] = sum_j w[j] * pref[b,j,:]

Strategy: pure data parallel over 8 NeuronCores (1024 batches each).
Per core, batches are processed in groups of GROUP (= NTILES tiles of 2
batches; a tile is the 128 flattened (batch, j) rows x 128 D columns).

Engine plan per tile (fp16 on the TensorEngine, fp32 elsewhere):
  - PE transpose pref_tile -> pref_T (D on partitions)
  - PE matmul  lhsT=pref_T,  rhs=c_T[:, 2t:2t+2]  -> per-row dots (psum)
  - PE matmul  lhsT=pref,    rhs=W_MAT[:, t, :]   -> u pair (psum)
Softmax / t_w run on VectorE/ScalarE in a "pair-major" layout
(partition = batch-pair, free = (parity, j)).
"""

import numpy as np
from contextlib import ExitStack

import concourse.bass as bass
import concourse.tile as tile
from concourse import mybir
from concourse.masks import make_identity
from concourse.bass_utils import run_bass_kernel_spmd
import concourse.bass2jax as _b2j


def _split_multiwait(bir: dict) -> int:
    """Walrus in this container rejects >1 sync-wait per instruction.

    Hoist excess waits onto NoOps inserted just before the instruction on
    the same engine (program order within the engine stream preserves the
    wait semantics exactly).
    """
    n = 0
    for fn in bir["functions"]:
        for blk in fn["blocks"]:
            out = []
            for inst in blk["instructions"]:
                si = inst.get("sync_info")
                waits = si.get("on_wait") if si else None
                if waits and len(waits) > 1:
                    for w in waits[:-1]:
                        out.append(
                            {
                                "opcode": "NoOp",
                                "engine": inst["engine"],
                                "name": f"{inst['name']}-xw{n}",
                                "ins": [],
                                "outs": [],
                                "sync_info": {"on_update": [], "on_wait": [w]},
                            }
                        )
                        n += 1
                    si["on_wait"] = [waits[-1]]
                out.append(inst)
            blk["instructions"] = out
    return n


_orig_compile_bir_kernel = _b2j.compile_bir_kernel


def _legalizing_compile_bir_kernel(ant_bir_str, *args, **kwargs):
    import orjson

    bir = orjson.loads(ant_bir_str)
    _split_multiwait(bir)
    return _orig_compile_bir_kernel(orjson.dumps(bir), *args, **kwargs)


_b2j.compile_bir_kernel = _legalizing_compile_bir_kernel

F32 = mybir.dt.float32
F16 = mybir.dt.float16
Alu = mybir.AluOpType
Act = mybir.ActivationFunctionType
Axis = mybir.AxisListType

B, N, D = 8192, 64, 128
NCORES = 8
BPC = B // NCORES          # 1024 batches per core
GROUP = 128               # batches per group
NGROUPS = BPC // GROUP     # 16
NTILES = GROUP // 2        # 32 two-batch tiles per group
NPAIR = GROUP // 2         # batch-pairs per group (softmax partitions)


def _build_group(tc, pools, consts, aps, g):
    nc = tc.nc
    (pref_rows, u_all, c32a, tpa, tca) = aps
    ident16, ident32 = consts
    (p_p32, p_p16, p_pt, p_small, ps_pt, ps_mm, ps_small) = pools

    r0 = g * GROUP * N          # first flat row of this group
    b0 = g * GROUP

    # ---- c transpose + t_w ----------------------------------------------
    cg16 = p_small.tile([GROUP, D], F16, tag="cg16")
    nc.vector.tensor_copy(out=cg16[:], in_=c32a[:, g, :])
    ct_ps = ps_small.tile([128, GROUP], F16, tag="sm_ps")
    nc.tensor.transpose(
        out=ct_ps[:], in_=cg16[:], identity=ident16[0:GROUP, 0:GROUP]
    )
    ct16 = p_small.tile([128, GROUP], F16, tag="ct16")   # [D, batch-in-group]
    nc.vector.tensor_copy(out=ct16[:], in_=ct_ps[:])

    tw = p_small.tile([NPAIR, 2, N], F32, tag="tw")
    for s in range(2):
        nc.vector.tensor_scalar_sub(
            out=tw[:, s, :], in0=tpa[:, g, s, :], scalar1=tca[:, g, s : s + 1]
        )
    nc.scalar.activation(out=tw[:], in_=tw[:], func=Act.Abs)
    nc.vector.reciprocal(out=tw[:], in_=tw[:])

    # ---- pref load with cast-in-DMA (SWDGE), in 8-tile chunks ------------
    HT = 8
    p16 = p_p16.tile([128, NTILES, D], F16, tag="p16")
    for h0 in range(0, NTILES, HT):
        rh = r0 + h0 * 128
        nc.gpsimd.dma_start(
            out=p16[:, h0 : h0 + HT, :],
            in_=pref_rows[rh : rh + HT * 128, :].rearrange(
                "(t p) d -> p t d", p=128
            ),
        )

    # ---- transposes + dot matmuls ---------------------------------------
    pts = p_pt.tile([128, NTILES, 128], F16, tag="pts")
    CH = 8
    for t0 in range(0, NTILES, CH):
        pt_ps = ps_pt.tile([128, CH, 128], F16, tag="pt_ps")
        for k in range(CH):
            nc.tensor.transpose(
                out=pt_ps[:, k, :], in_=p16[:, t0 + k, :], identity=ident16[:]
            )
        if (t0 // CH) % 3 == 2:
            nc.vector.tensor_copy(out=pts[:, t0 : t0 + CH, :], in_=pt_ps[:])
        else:
            nc.scalar.copy(out=pts[:, t0 : t0 + CH, :], in_=pt_ps[:])

    ps_dots = ps_mm.tile([128, NTILES, 2], F32, tag="mm_ps")
    for t in range(NTILES):
        nc.tensor.matmul(
            out=ps_dots[:, t, :],
            lhsT=pts[:, t, :],
            rhs=ct16[:, 2 * t : 2 * t + 2],
            start=(t == 0),
            stop=(t == NTILES - 1),
        )

    # valid dots sit at [row, parity=row//64]: extract the two halves
    dotw_rows = p_small.tile([128, NTILES], F32, tag="dotw_rows")
    nc.scalar.copy(out=dotw_rows[0:64, :], in_=ps_dots[0:64, :, 0])
    nc.scalar.copy(out=dotw_rows[64:128, :], in_=ps_dots[64:128, :, 1])

    # transpose [128(row), nt] -> [nt, 128(row)] => pair-major dots
    dr_ps = ps_small.tile([NPAIR, 128], F32, tag="sm_ps")
    nc.tensor.transpose(out=dr_ps[:], in_=dotw_rows[:], identity=ident32[:])

    # ---- softmax over j (segmented, pair-major) --------------------------
    w = p_small.tile([NPAIR, 2, N], F32, tag="w")
    nc.vector.tensor_add(
        out=w[:],
        in0=dr_ps[:].rearrange("t (two n) -> t two n", two=2),
        in1=tw[:],
    )
    nmx = p_small.tile([NPAIR, 2], F32, tag="nmx")
    nc.vector.tensor_reduce(
        out=nmx[:], in_=w[:], axis=Axis.X, op=Alu.max, negate=True
    )
    e = p_small.tile([NPAIR, 2, N], F32, tag="e")
    for s in range(2):
        nc.scalar.activation(
            out=e[:, s, :],
            in_=w[:, s, :],
            func=Act.Exp,
            bias=nmx[:, s : s + 1],
            scale=1.0,
        )
    z = p_small.tile([NPAIR, 2], F32, tag="z")
    nc.vector.reduce_sum(out=z[:], in_=e[:], axis=Axis.X)
    rz = p_small.tile([NPAIR, 2], F32, tag="rz")
    nc.vector.reciprocal(out=rz[:], in_=z[:])
    wn16 = p_small.tile([NPAIR, 2, N], F16, tag="wn16")
    for s in range(2):
        nc.vector.tensor_scalar_mul(
            out=wn16[:, s, :], in0=e[:, s, :], scalar1=rz[:, s : s + 1]
        )

    # ---- build W_MAT [row, t, parity] (block structure, zeros elsewhere) --
    wc_ps = ps_small.tile([128, NTILES], F16, tag="sm_ps")
    nc.tensor.transpose(
        out=wc_ps[:],
        in_=wn16[:].rearrange("t two n -> t (two n)"),
        identity=ident16[0:NPAIR, 0:NPAIR],
    )
    wcol16 = p_small.tile([128, NTILES], F16, tag="wcol16")
    nc.vector.tensor_copy(out=wcol16[:], in_=wc_ps[:])
    wmat16 = p_small.tile([128, NTILES, 2], F16, tag="wmat16")
    nc.vector.memset(wmat16[:], 0.0)
    nc.vector.tensor_copy(out=wmat16[0:64, :, 0], in_=wcol16[0:64, :])
    nc.vector.tensor_copy(out=wmat16[64:128, :, 1], in_=wcol16[64:128, :])

    # ---- weighted-sum matmuls + store, in half-groups --------------------
    HB = NTILES // 2
    for h in range(2):
        ps_ut = ps_mm.tile([128, HB, 2], F32, tag="mm_ps")
        for k in range(HB):
            t = h * HB + k
            nc.tensor.matmul(
                out=ps_ut[:, k, :],
                lhsT=p16[:, t, :],
                rhs=wmat16[:, t, :],
                start=(k == 0),
                stop=(k == HB - 1),
            )
        uts = p_small.tile([128, GROUP // 2], F32, tag="uts")
        nc.vector.tensor_copy(
            out=uts[:], in_=ps_ut[:].rearrange("d t two -> d (t two)")
        )
        ug_ps = ps_small.tile([GROUP // 2, 128], F32, tag="sm_ps")
        nc.tensor.transpose(out=ug_ps[:], in_=uts[:], identity=ident32[:])
        ug = p_small.tile([GROUP // 2, 128], F32, tag="ug")
        nc.vector.tensor_copy(out=ug[:], in_=ug_ps[:])
        bh = b0 + h * (GROUP // 2)
        nc.gpsimd.dma_start(
            out=u_all[bh : bh + GROUP // 2, :].rearrange(
                "b (x d) -> b x d", x=2
            ),
            in_=ug[:].rearrange("b (x d) -> b x d", x=2),
        )


def _build_nc():
    nc = bass.Bass()
    pref = nc.declare_dram_parameter("pref", [BPC, N, D], F32, isOutput=False)
    c = nc.declare_dram_parameter("c", [BPC, 1, D], F32, isOutput=False)
    t_pref = nc.declare_dram_parameter("t_pref", [BPC, 1, N], F32, isOutput=False)
    t_c = nc.declare_dram_parameter("t_c", [BPC, 1], F32, isOutput=False)
    u = nc.declare_dram_parameter("u", [BPC, 1, D], F32, isOutput=True)

    pref_rows = pref[:].rearrange("b n d -> (b n) d")
    c_all = c[:].rearrange("b one d -> (b one) d")
    tp_all = t_pref[:].rearrange("b one n -> (b one) n")
    tc_all = t_c[:]
    u_all = u[:].rearrange("b one d -> (b one) d")

    with ExitStack() as ctx:
        tc = ctx.enter_context(tile.TileContext(nc))
        p_const = ctx.enter_context(tc.tile_pool(name="const", bufs=1))
        ident16 = p_const.tile([128, 128], F16)
        ident32 = p_const.tile([128, 128], F32)
        make_identity(nc, ident16[:])
        make_identity(nc, ident32[:])
        consts = (ident16, ident32)

        p_pre = ctx.enter_context(tc.tile_pool(name="pre", bufs=1))
        nb = NGROUPS * GROUP
        c32a = p_pre.tile([GROUP, NGROUPS, D], F32)
        nc.sync.dma_start(
            out=c32a[:],
            in_=c_all[0:nb, :].rearrange("(g b) d -> b g d", b=GROUP),
        )
        tpa = p_pre.tile([NPAIR, NGROUPS, 2, N], F32)
        nc.sync.dma_start(
            out=tpa[:],
            in_=tp_all[0:nb, :].rearrange(
                "(g t two) n -> t g two n", t=NPAIR, two=2
            ),
        )
        tca = p_pre.tile([NPAIR, NGROUPS, 2], F32)
        nc.sync.dma_start(
            out=tca[:],
            in_=tc_all[0:nb, :].rearrange(
                "(g t two) one -> t g (two one)", t=NPAIR, two=2
            ),
        )
        aps = (pref_rows, u_all, c32a, tpa, tca)

        p_p32 = ctx.enter_context(tc.tile_pool(name="p32", bufs=6))
        p_p16 = ctx.enter_context(tc.tile_pool(name="p16", bufs=5))
        p_pt = ctx.enter_context(tc.tile_pool(name="pt", bufs=4))
        p_small = ctx.enter_context(tc.tile_pool(name="small", bufs=3))
        ps_pt = ctx.enter_context(tc.tile_pool(name="ps_pt", bufs=3, space="PSUM"))
        ps_mm = ctx.enter_context(tc.tile_pool(name="ps_mm", bufs=2, space="PSUM"))
        ps_small = ctx.enter_context(
            tc.tile_pool(name="ps_small", bufs=3, space="PSUM")
        )
        pools = (p_p32, p_p16, p_pt, p_small, ps_pt, ps_mm, ps_small)

        for g in range(NGROUPS):
            _build_group(tc, pools, consts, aps, g)

    return nc


_NC_CACHE = None
LAST_RESULT = None


def kernel(pref, c, t_pref, t_c):
    global _NC_CACHE, LAST_RESULT
    if _NC_CACHE is None:
        _NC_CACHE = _build_nc()
    nc = _NC_CACHE

    pref = np.ascontiguousarray(pref, dtype=np.float32)
    c = np.ascontiguousarray(c, dtype=np.float32)
    t_pref = np.ascontiguousarray(t_pref, dtype=np.float32)
    t_c = np.ascontiguousarray(t_c, dtype=np.float32)

    in_maps = []
    for i in range(NCORES):
        s = slice(i * BPC, (i + 1) * BPC)
        in_maps.append(
            {"pref": pref[s], "c": c[s], "t_pref": t_pref[s], "t_c": t_c[s]}
        )

    res = run_bass_kernel_spmd(nc, in_maps, list(range(NCORES)))
    LAST_RESULT = res
    return np.concatenate([r["u"] for r in res.results], axis=0)



# revision 52
# speedup vs baseline: 1.0167x; 1.0167x over previous
"""Trainium2 Bass kernel for the AggregateLayer pooling problem.

reference semantics (per batch b):
    dot_w[j] = <pref[b,j,:], c[b,0,:]>                      (j = 0..63)
    t_w[j]   = 1 / |t_pref[b,0,j] - t_c[b,0]|
    w        = softmax(dot_w + t_w)                          (over j)
    u[b,0,:] = sum_j w[j] * pref[b,j,:]

Strategy: pure data parallel over 8 NeuronCores (1024 batches each).
Per core, batches are processed in groups of GROUP (= NTILES tiles of 2
batches; a tile is the 128 flattened (batch, j) rows x 128 D columns).

Engine plan per tile (fp16 on the TensorEngine, fp32 elsewhere):
  - PE transpose pref_tile -> pref_T (D on partitions)
  - PE matmul  lhsT=pref_T,  rhs=c_T[:, 2t:2t+2]  -> per-row dots (psum)
  - PE matmul  lhsT=pref,    rhs=W_MAT[:, t, :]   -> u pair (psum)
Softmax / t_w run on VectorE/ScalarE in a "pair-major" layout
(partition = batch-pair, free = (parity, j)).
"""

import numpy as np
from contextlib import ExitStack

import concourse.bass as bass
import concourse.tile as tile
from concourse import mybir
from concourse.masks import make_identity
from concourse.bass_utils import run_bass_kernel_spmd
import concourse.bass2jax as _b2j


def _split_multiwait(bir: dict) -> int:
    """Walrus in this container rejects >1 sync-wait per instruction.

    Hoist excess waits onto NoOps inserted just before the instruction on
    the same engine (program order within the engine stream preserves the
    wait semantics exactly).
    """
    n = 0
    for fn in bir["functions"]:
        for blk in fn["blocks"]:
            out = []
            for inst in blk["instructions"]:
                si = inst.get("sync_info")
                waits = si.get("on_wait") if si else None
                if waits and len(waits) > 1:
                    for w in waits[:-1]:
                        out.append(
                            {
                                "opcode": "NoOp",
                                "engine": inst["engine"],
                                "name": f"{inst['name']}-xw{n}",
                                "ins": [],
                                "outs": [],
                                "sync_info": {"on_update": [], "on_wait": [w]},
                            }
                        )
                        n += 1
                    si["on_wait"] = [waits[-1]]
                out.append(inst)
            blk["instructions"] = out
    return n


_orig_compile_bir_kernel = _b2j.compile_bir_kernel


def _legalizing_compile_bir_kernel(ant_bir_str, *args, **kwargs):
    import orjson

    bir = orjson.loads(ant_bir_str)
    _split_multiwait(bir)
    return _orig_compile_bir_kernel(orjson.dumps(bir), *args, **kwargs)


_b2j.compile_bir_kernel = _legalizing_compile_bir_kernel

F32 = mybir.dt.float32
F16 = mybir.dt.float16
Alu = mybir.AluOpType
Act = mybir.ActivationFunctionType
Axis = mybir.AxisListType

B, N, D = 8192, 64, 128
NCORES = 8
BPC = B // NCORES          # 1024 batches per core
GROUP = 128               # batches per group
NGROUPS = BPC // GROUP     # 16
NTILES = GROUP // 2        # 32 two-batch tiles per group
NPAIR = GROUP // 2         # batch-pairs per group (softmax partitions)


def _build_group(tc, pools, consts, aps, g):
    nc = tc.nc
    (pref_rows, u_all, c32a, tpa, tca) = aps
    ident16, ident32 = consts
    (p_p32, p_p16, p_pt, p_small, ps_pt, ps_mm, ps_small) = pools

    r0 = g * GROUP * N          # first flat row of this group
    b0 = g * GROUP

    # ---- c transpose + t_w ----------------------------------------------
    cg16 = p_small.tile([GROUP, D], F16, tag="cg16")
    nc.vector.tensor_copy(out=cg16[:], in_=c32a[:, g, :])
    ct_ps = ps_small.tile([128, GROUP], F16, tag="sm_ps")
    nc.tensor.transpose(
        out=ct_ps[:], in_=cg16[:], identity=ident16[0:GROUP, 0:GROUP]
    )
    ct16 = p_small.tile([128, GROUP], F16, tag="ct16")   # [D, batch-in-group]
    nc.vector.tensor_copy(out=ct16[:], in_=ct_ps[:])

    tw = p_small.tile([NPAIR, 2, N], F32, tag="tw")
    for s in range(2):
        nc.vector.tensor_scalar_sub(
            out=tw[:, s, :], in0=tpa[:, g, s, :], scalar1=tca[:, g, s : s + 1]
        )
    nc.scalar.activation(out=tw[:], in_=tw[:], func=Act.Abs)
    nc.vector.reciprocal(out=tw[:], in_=tw[:])

    # ---- pref load with cast-in-DMA (SWDGE), in 8-tile chunks ------------
    HT = 8
    p16 = p_p16.tile([128, NTILES, D], F16, tag="p16")
    for h0 in range(0, NTILES, HT):
        rh = r0 + h0 * 128
        nc.gpsimd.dma_start(
            out=p16[:, h0 : h0 + HT, :],
            in_=pref_rows[rh : rh + HT * 128, :].rearrange(
                "(t p) d -> p t d", p=128
            ),
        )

    # ---- transposes + dot matmuls ---------------------------------------
    pts = p_pt.tile([128, NTILES, 128], F16, tag="pts")
    CH = 8
    for t0 in range(0, NTILES, CH):
        pt_ps = ps_pt.tile([128, CH, 128], F16, tag="pt_ps")
        for k in range(CH):
            nc.tensor.transpose(
                out=pt_ps[:, k, :], in_=p16[:, t0 + k, :], identity=ident16[:]
            )
        if (t0 // CH) % 3 == 2:
            nc.vector.tensor_copy(out=pts[:, t0 : t0 + CH, :], in_=pt_ps[:])
        else:
            nc.scalar.copy(out=pts[:, t0 : t0 + CH, :], in_=pt_ps[:])

    ps_dots = ps_mm.tile([128, NTILES, 2], F32, tag="mm_ps")
    for t in range(NTILES):
        nc.tensor.matmul(
            out=ps_dots[:, t, :],
            lhsT=pts[:, t, :],
            rhs=ct16[:, 2 * t : 2 * t + 2],
            start=(t == 0),
            stop=(t == NTILES - 1),
        )

    # valid dots sit at [row, parity=row//64]: extract the two halves
    dotw_rows = p_small.tile([128, NTILES], F32, tag="dotw_rows")
    nc.scalar.copy(out=dotw_rows[0:64, :], in_=ps_dots[0:64, :, 0])
    nc.scalar.copy(out=dotw_rows[64:128, :], in_=ps_dots[64:128, :, 1])

    # transpose [128(row), nt] -> [nt, 128(row)] => pair-major dots
    dr_ps = ps_small.tile([NPAIR, 128], F32, tag="sm_ps")
    nc.tensor.transpose(out=dr_ps[:], in_=dotw_rows[:], identity=ident32[:])

    # ---- softmax over j (segmented, pair-major) --------------------------
    w = p_small.tile([NPAIR, 2, N], F32, tag="w")
    nc.vector.tensor_add(
        out=w[:],
        in0=dr_ps[:].rearrange("t (two n) -> t two n", two=2),
        in1=tw[:],
    )
    nmx = p_small.tile([NPAIR, 2], F32, tag="nmx")
    nc.vector.tensor_reduce(
        out=nmx[:], in_=w[:], axis=Axis.X, op=Alu.max, negate=True
    )
    e = p_small.tile([NPAIR, 2, N], F32, tag="e")
    for s in range(2):
        nc.scalar.activation(
            out=e[:, s, :],
            in_=w[:, s, :],
            func=Act.Exp,
            bias=nmx[:, s : s + 1],
            scale=1.0,
        )
    z = p_small.tile([NPAIR, 2], F32, tag="z")
    nc.vector.reduce_sum(out=z[:], in_=e[:], axis=Axis.X)
    rz = p_small.tile([NPAIR, 2], F32, tag="rz")
    nc.vector.reciprocal(out=rz[:], in_=z[:])
    wn16 = p_small.tile([NPAIR, 2, N], F16, tag="wn16")
    for s in range(2):
        nc.vector.tensor_scalar_mul(
            out=wn16[:, s, :], in0=e[:, s, :], scalar1=rz[:, s : s + 1]
        )

    # ---- build W_MAT [row, t, parity] (block structure, zeros elsewhere) --
    wc_ps = ps_small.tile([128, NTILES], F16, tag="sm_ps")
    nc.tensor.transpose(
        out=wc_ps[:],
        in_=wn16[:].rearrange("t two n -> t (two n)"),
        identity=ident16[0:NPAIR, 0:NPAIR],
    )
    wcol16 = p_small.tile([128, NTILES], F16, tag="wcol16")
    nc.vector.tensor_copy(out=wcol16[:], in_=wc_ps[:])
    wmat16 = p_small.tile([128, NTILES, 2], F16, tag="wmat16")
    nc.vector.memset(wmat16[:], 0.0)
    nc.vector.tensor_copy(out=wmat16[0:64, :, 0], in_=wcol16[0:64, :])
    nc.vector.tensor_copy(out=wmat16[64:128, :, 1], in_=wcol16[64:128, :])

    # ---- weighted-sum matmuls + store, in half-groups --------------------
    HB = NTILES // 2
    for h in range(2):
        ps_ut = ps_mm.tile([128, HB, 2], F32, tag="mm_ps")
        for k in range(HB):
            t = h * HB + k
            nc.tensor.matmul(
                out=ps_ut[:, k, :],
                lhsT=p16[:, t, :],
                rhs=wmat16[:, t, :],
                start=(k == 0),
                stop=(k == HB - 1),
            )
        uts = p_small.tile([128, GROUP // 2], F32, tag="uts")
        nc.vector.tensor_copy(
            out=uts[:], in_=ps_ut[:].rearrange("d t two -> d (t two)")
        )
        ug_ps = ps_small.tile([GROUP // 2, 128], F32, tag="sm_ps")
        nc.tensor.transpose(out=ug_ps[:], in_=uts[:], identity=ident32[:])
        ug = p_small.tile([GROUP // 2, 128], F32, tag="ug")
        nc.vector.tensor_copy(out=ug[:], in_=ug_ps[:])
        bh = b0 + h * (GROUP // 2)
        nc.gpsimd.dma_start(
            out=u_all[bh : bh + GROUP // 2, :].rearrange(
                "b (x d) -> b x d", x=2
            ),
            in_=ug[:].rearrange("b (x d) -> b x d", x=2),
        )


def _build_nc():
    nc = bass.Bass()
    pref = nc.declare_dram_parameter("pref", [BPC, N, D], F32, isOutput=False)
    c = nc.declare_dram_parameter("c", [BPC, 1, D], F32, isOutput=False)
    t_pref = nc.declare_dram_parameter("t_pref", [BPC, 1, N], F32, isOutput=False)
    t_c = nc.declare_dram_parameter("t_c", [BPC, 1], F32, isOutput=False)
    u = nc.declare_dram_parameter("u", [BPC, 1, D], F32, isOutput=True)

    pref_rows = pref[:].rearrange("b n d -> (b n) d")
    c_all = c[:].rearrange("b one d -> (b one) d")
    tp_all = t_pref[:].rearrange("b one n -> (b one) n")
    tc_all = t_c[:]
    u_all = u[:].rearrange("b one d -> (b one) d")

    with ExitStack() as ctx:
        tc = ctx.enter_context(tile.TileContext(nc))
        p_const = ctx.enter_context(tc.tile_pool(name="const", bufs=1))
        ident16 = p_const.tile([128, 128], F16)
        ident32 = p_const.tile([128, 128], F32)
        make_identity(nc, ident16[:])
        make_identity(nc, ident32[:])
        consts = (ident16, ident32)

        p_pre = ctx.enter_context(tc.tile_pool(name="pre", bufs=1))
        nb = NGROUPS * GROUP
        c32a = p_pre.tile([GROUP, NGROUPS, D], F32)
        nc.sync.dma_start(
            out=c32a[:],
            in_=c_all[0:nb, :].rearrange("(g b) d -> b g d", b=GROUP),
        )
        tpa = p_pre.tile([NPAIR, NGROUPS, 2, N], F32)
        nc.sync.dma_start(
            out=tpa[:],
            in_=tp_all[0:nb, :].rearrange(
                "(g t two) n -> t g two n", t=NPAIR, two=2
            ),
        )
        tca = p_pre.tile([NPAIR, NGROUPS, 2], F32)
        nc.sync.dma_start(
            out=tca[:],
            in_=tc_all[0:nb, :].rearrange(
                "(g t two) one -> t g (two one)", t=NPAIR, two=2
            ),
        )
        aps = (pref_rows, u_all, c32a, tpa, tca)

        p_p32 = ctx.enter_context(tc.tile_pool(name="p32", bufs=6))
        p_p16 = ctx.enter_context(tc.tile_pool(name="p16", bufs=5))
        p_pt = ctx.enter_context(tc.tile_pool(name="pt", bufs=5))
        p_small = ctx.enter_context(tc.tile_pool(name="small", bufs=3))
        ps_pt = ctx.enter_context(tc.tile_pool(name="ps_pt", bufs=3, space="PSUM"))
        ps_mm = ctx.enter_context(tc.tile_pool(name="ps_mm", bufs=2, space="PSUM"))
        ps_small = ctx.enter_context(
            tc.tile_pool(name="ps_small", bufs=3, space="PSUM")
        )
        pools = (p_p32, p_p16, p_pt, p_small, ps_pt, ps_mm, ps_small)

        for g in range(NGROUPS):
            _build_group(tc, pools, consts, aps, g)

    return nc


_NC_CACHE = None
LAST_RESULT = None


def kernel(pref, c, t_pref, t_c):
    global _NC_CACHE, LAST_RESULT
    if _NC_CACHE is None:
        _NC_CACHE = _build_nc()
    nc = _NC_CACHE

    pref = np.ascontiguousarray(pref, dtype=np.float32)
    c = np.ascontiguousarray(c, dtype=np.float32)
    t_pref = np.ascontiguousarray(t_pref, dtype=np.float32)
    t_c = np.ascontiguousarray(t_c, dtype=np.float32)

    in_maps = []
    for i in range(NCORES):
        s = slice(i * BPC, (i + 1) * BPC)
        in_maps.append(
            {"pref": pref[s], "c": c[s], "t_pref": t_pref[s], "t_c": t_c[s]}
        )

    res = run_bass_kernel_spmd(nc, in_maps, list(range(NCORES)))
    LAST_RESULT = res
    return np.concatenate([r["u"] for r in res.results], axis=0)

